# revision 2
# baseline (speedup 1.0000x reference)
"""Bass/Trainium2 kernel for nn_Attention_Layer (B=8, L=2048, D=1024).

Strategy (V1): pure data-parallel over batch — core c computes the full
attention layer for batch element c.

Per-core pipeline (everything on-chip after one load of x^T and weights):
  1. Projections on TensorE (fp16 in, fp32 PSUM accum):
       QT[e,l] = WqT.T @ xT   (lhsT = WqT[d,e] tile, rhs = xT[d,l])  -> fp16
       KT[e,l] likewise                                              -> fp16
       V[l,e]  = xT.T @ WvT   (lhsT = xT[d,l] tile, rhs = WvT[d,e])  -> bf16
  2. Scores transposed: ST[k,q] = KT.T @ QT (contract over e).
     Masking+softmax numerator fused into one ScalarE activation:
       E[k,q] = exp(ST[k,q] + bias[k]),  bias[k] = -44 (valid) / -1e30 (masked)
     The -44 shift keeps exp in comfortable fp32 range (scores reach ~±65);
     it cancels in U/r. E stored bf16 (full fp32 exponent range).
  3. U[q,e] = E.T @ V and r[q] = E.T @ ones (contract over k, on TensorE).
  4. out[q,e] = U[q,e] * (1/r[q]) on VectorE; DMA out fp32.

No per-row max subtraction is needed: scores are O(60) so exp stays finite
in fp32, and the reference's -2^31 padding value is reproduced exactly by
the additive -1e30 mask (exp -> 0).
"""

import os

import numpy as np

import concourse.bass as bass
import concourse.tile as tile
import concourse.bacc as bacc
from concourse import mybir
from concourse.bass_utils import run_bass_kernel_spmd

B, L, D = 8, 2048, 1024
P = 128
NDT = D // P   # 8 d-tiles (contraction tiles for projections)
NET = D // P   # 8 e-tiles (feature tiles)
NKT = L // P   # 16 k-tiles (key tiles)
NQT = L // P   # 16 q-tiles
QB = 512       # q-block width for the score matmuls
NQB = L // QB  # 4
MASK_SHIFT = -44.0
MASK_NEG = -1.0e30

f16 = mybir.dt.float16
bf16 = mybir.dt.bfloat16
f32 = mybir.dt.float32

LAST_RESULT = None
_NC_CACHE = {}


def _build_v1():
    nc = bacc.Bacc("TRN2", target_bir_lowering=False, debug=False, num_devices=B)

    xT_d = nc.dram_tensor("xT", [D, L], f16, kind="ExternalInput").ap()
    wqT_d = nc.dram_tensor("wqT", [D, D], f16, kind="ExternalInput").ap()
    wkT_d = nc.dram_tensor("wkT", [D, D], f16, kind="ExternalInput").ap()
    wvT_d = nc.dram_tensor("wvT", [D, D], f16, kind="ExternalInput").ap()
    maskT_d = nc.dram_tensor("maskT", [P, NKT], f32, kind="ExternalInput").ap()
    out_d = nc.dram_tensor("out", [L, D], f32, kind="ExternalOutput").ap()

    Exp = mybir.ActivationFunctionType.Exp

    with tile.TileContext(nc) as tc:
        with tc.tile_pool(name="qkv", bufs=1) as qkv_pool, \
             tc.tile_pool(name="cst", bufs=1) as cst_pool:
            # Long-lived tensors for the attention phase.
            QT = [qkv_pool.tile([P, L], f16, name=f"QT{i}", tag=f"QT{i}") for i in range(NET)]
            KT = [qkv_pool.tile([P, L], f16, name=f"KT{i}", tag=f"KT{i}") for i in range(NET)]
            V = [qkv_pool.tile([P, D], bf16, name=f"V{i}", tag=f"V{i}") for i in range(NKT)]
            maskT = cst_pool.tile([P, NKT], f32, name="maskT", tag="maskT")
            ones = cst_pool.tile([P, 1], bf16, name="ones", tag="ones")
            nc.sync.dma_start(maskT[:], maskT_d[:, :])
            nc.vector.memset(ones[:], 1.0)

            # ---- Phase 1: projections ----
            with tc.tile_pool(name="xw", bufs=1) as xw_pool, \
                 tc.tile_pool(name="pproj", bufs=4, space="PSUM") as pproj:
                xT = [xw_pool.tile([P, L], f16, name=f"xT{i}", tag=f"xT{i}") for i in range(NDT)]
                wq = [xw_pool.tile([P, D], f16, name=f"wq{i}", tag=f"wq{i}") for i in range(NDT)]
                wk = [xw_pool.tile([P, D], f16, name=f"wk{i}", tag=f"wk{i}") for i in range(NDT)]
                wv = [xw_pool.tile([P, D], f16, name=f"wv{i}", tag=f"wv{i}") for i in range(NDT)]
                for i in range(NDT):
                    sl = slice(i * P, (i + 1) * P)
                    nc.sync.dma_start(xT[i][:], xT_d[sl, :])
                    nc.sync.dma_start(wq[i][:], wqT_d[sl, :])
                    nc.sync.dma_start(wk[i][:], wkT_d[sl, :])
                    nc.sync.dma_start(wv[i][:], wvT_d[sl, :])

                # QT / KT: out[e-tile, l-block]
                for w_t, dstT in ((wq, QT), (wk, KT)):
                    for et in range(NET):
                        for lb in range(L // QB):
                            ps = pproj.tile([P, QB], f32, name="pp", tag="pp")
                            for dt_ in range(NDT):
                                nc.tensor.matmul(
                                    ps[:],
                                    lhsT=w_t[dt_][:, et * P:(et + 1) * P],
                                    rhs=xT[dt_][:, lb * QB:(lb + 1) * QB],
                                    start=(dt_ == 0), stop=(dt_ == NDT - 1),
                                )
                            nc.vector.tensor_copy(
                                dstT[et][:, lb * QB:(lb + 1) * QB], ps[:])
                # V: out[l-tile, e-block]
                for lt in range(NQT):
                    for eb in range(D // QB):
                        ps = pproj.tile([P, QB], f32, name="pp", tag="pp")
                        for dt_ in range(NDT):
                            nc.tensor.matmul(
                                ps[:],
                                lhsT=xT[dt_][:, lt * P:(lt + 1) * P],
                                rhs=wv[dt_][:, eb * QB:(eb + 1) * QB],
                                start=(dt_ == 0), stop=(dt_ == NDT - 1),
                            )
                        nc.vector.tensor_copy(
                            V[lt][:, eb * QB:(eb + 1) * QB], ps[:])

            # ---- Phase 2: attention ----
            with tc.tile_pool(name="attn", bufs=2) as attn_pool, \
                 tc.tile_pool(name="outp", bufs=3) as outp, \
                 tc.tile_pool(name="small", bufs=4) as small, \
                 tc.tile_pool(name="ps_s", bufs=2, space="PSUM") as ps_s, \
                 tc.tile_pool(name="ps_u", bufs=2, space="PSUM") as ps_u, \
                 tc.tile_pool(name="ps_r", bufs=2, space="PSUM") as ps_r:
                for qb in range(NQB):
                    qsl = slice(qb * QB, (qb + 1) * QB)
                    E = attn_pool.tile([P, NKT, QB], bf16, name="E", tag="E")
                    for kt in range(NKT):
                        ps = ps_s.tile([P, QB], f32, name="ps", tag="ps")
                        for et in range(NET):
                            nc.tensor.matmul(
                                ps[:],
                                lhsT=KT[et][:, kt * P:(kt + 1) * P],
                                rhs=QT[et][:, qsl],
                                start=(et == 0), stop=(et == NET - 1),
                            )
                        nc.scalar.activation(
                            E[:, kt, :], ps[:], Exp,
                            bias=maskT[:, kt:kt + 1], scale=1.0)
                    for qt in range(QB // P):
                        q0 = qb * QB + qt * P  # global q row start
                        psU = ps_u.tile([P, D], f32, name="psU", tag="psU")
                        psr = ps_r.tile([P, 1], f32, name="psr", tag="psr")
                        for kt in range(NKT):
                            lhsT = E[:, kt, qt * P:(qt + 1) * P]
                            st, sp = (kt == 0), (kt == NKT - 1)
                            nc.tensor.matmul(psU[:, 0:QB], lhsT=lhsT,
                                             rhs=V[kt][:, 0:QB],
                                             start=st, stop=sp)
                            nc.tensor.matmul(psU[:, QB:D], lhsT=lhsT,
                                             rhs=V[kt][:, QB:D],
                                             start=st, stop=sp)
                            nc.tensor.matmul(psr[:], lhsT=lhsT, rhs=ones[:],
                                             start=st, stop=sp)
                        rinv = small.tile([P, 1], f32, name="rinv", tag="rinv")
                        nc.vector.reciprocal(rinv[:], psr[:])
                        ob = outp.tile([P, D], f32, name="ob", tag="ob")
                        nc.vector.tensor_scalar_mul(ob[:, 0:QB], psU[:, 0:QB], rinv[:])
                        nc.vector.tensor_scalar_mul(ob[:, QB:D], psU[:, QB:D], rinv[:])
                        nc.sync.dma_start(out_d[q0:q0 + P, :], ob[:])

    nc.compile()
    return nc


def _get_nc():
    if "v1" not in _NC_CACHE:
        _NC_CACHE["v1"] = _build_v1()
    return _NC_CACHE["v1"]


def kernel(inputs, Wq, Wk, Wv, lens):
    global LAST_RESULT
    inputs = np.asarray(inputs, dtype=np.float32)
    Wq = np.asarray(Wq, dtype=np.float32)
    Wk = np.asarray(Wk, dtype=np.float32)
    Wv = np.asarray(Wv, dtype=np.float32)
    lens = np.asarray(lens, dtype=np.int32)

    wqT = np.ascontiguousarray(Wq.T).astype(np.float16)
    wkT = np.ascontiguousarray(Wk.T).astype(np.float16)
    wvT = np.ascontiguousarray(Wv.T).astype(np.float16)

    ar = np.arange(L, dtype=np.int64)
    in_maps = []
    for c in range(B):
        xT = np.ascontiguousarray(inputs[c].T).astype(np.float16)
        mask = np.where(ar < int(lens[c]), MASK_SHIFT, MASK_NEG).astype(np.float32)
        maskT = np.ascontiguousarray(mask.reshape(NKT, P).T)  # [P, NKT]
        in_maps.append({
            "xT": xT, "wqT": wqT, "wkT": wkT, "wvT": wvT, "maskT": maskT,
        })

    nc = _get_nc()
    res = run_bass_kernel_spmd(nc, in_maps, core_ids=list(range(B)))
    LAST_RESULT = res
    out = np.stack([res.results[c]["out"] for c in range(B)], axis=0)
    return out.astype(np.float32)
